# revision 9
# baseline (speedup 1.0000x reference)
import sys

sys.path.insert(0, "/opt/trn_rl_repo")

import numpy as np

P = 128          # partitions / tile edge
D = 128          # model dim
H = 4            # heads
DH = 32          # head dim
NCORES = 8

# Full-problem geometry (N=100000, E=800000). Each core owns NBLK node
# blocks of 128 nodes; every block's incident-edge list is padded to
# TBLK tiles of 128 edges so the SPMD program is uniform across cores.
NBLK_FULL = 98                      # 98*128 = 12544 own nodes/core
NPAD_FULL = NCORES * NBLK_FULL * P  # 100352 padded nodes


def _channel_perm():
    # torch reshape (N, DH, H): flat channel c = d*H + h. We relayout to
    # h-major c' = h*DH + d by permuting weight rows: perm[c'] = d*H + h.
    cp = np.arange(D)
    return (cp % DH) * H + (cp // DH)


def _build_program(NPAD, NOWN, NBLK, TBLK):
    import concourse.bass as bass
    import concourse.tile as tile
    from concourse import bacc, mybir
    from concourse.masks import make_identity
    from contextlib import ExitStack

    dt = mybir.dt
    f32, f16, bf16, i32 = dt.float32, dt.float16, dt.bfloat16, dt.int32
    NT = NBLK * TBLK      # edge tiles per core
    XT = NPAD // P        # x tiles for k/v projection (all nodes)
    QT = NOWN // P        # x tiles for q projection (own nodes) == NBLK

    nc = bacc.Bacc("TRN2", target_bir_lowering=False, debug=False,
                   num_devices=NCORES)

    x_d = nc.dram_tensor("x", [NPAD, D], f32, kind="ExternalInput").ap()
    xo_d = nc.dram_tensor("xo", [NOWN, D], f32, kind="ExternalInput").ap()
    wkv_d = nc.dram_tensor("wkv", [D, 2 * D], f16, kind="ExternalInput").ap()
    wq_d = nc.dram_tensor("wq", [D, D], f16, kind="ExternalInput").ap()
    wo_d = nc.dram_tensor("wo", [D, D], f16, kind="ExternalInput").ap()
    bkv_d = nc.dram_tensor("bkv", [1, 2 * D], f16, kind="ExternalInput").ap()
    bq_d = nc.dram_tensor("bq", [1, D], f16, kind="ExternalInput").ap()
    bo_d = nc.dram_tensor("bo", [1, D], f16, kind="ExternalInput").ap()
    ci_d = nc.dram_tensor("ci", [P, NT], i32, kind="ExternalInput").ap()
    qi_d = nc.dram_tensor("qi", [P, NT], i32, kind="ExternalInput").ap()
    rl_d = nc.dram_tensor("rl", [P, NT], f16, kind="ExternalInput").ap()
    io_d = nc.dram_tensor("io", [P, P], f16, kind="ExternalInput").ap()

    out_d = nc.dram_tensor("out", [NOWN, D], f32, kind="ExternalOutput").ap()
    kv_d = nc.dram_tensor("kv", [NPAD, 2 * D], f16).ap()
    q_d = nc.dram_tensor("q", [NOWN, D], f16).ap()

    AF = mybir.ActivationFunctionType
    OP = mybir.AluOpType

    with tile.TileContext(nc) as tc, ExitStack() as ctx:
        res = ctx.enter_context(tc.tile_pool(name="res", bufs=1))
        wkv_sb = res.tile([D, 2 * D], f16, name="wkv_sb")
        wq_sb = res.tile([D, D], f16, name="wq_sb")
        wo_sb = res.tile([D, D], f16, name="wo_sb")
        bkv_sb = res.tile([1, 2 * D], f16, name="bkv_sb")
        bq_sb = res.tile([1, D], f16, name="bq_sb")
        bo_sb = res.tile([1, D], f16, name="bo_sb")
        ci_sb = res.tile([P, NT], i32, name="ci_sb")
        qi_sb = res.tile([P, NT], i32, name="qi_sb")
        rl_sb = res.tile([P, NT], f16, name="rl_sb")
        io_sb = res.tile([P, P], f16, name="io_sb")
        ones_sb = res.tile([1, P], f16, name="ones_sb")
        ident = res.tile([P, P], f16, name="ident")

        for sb_t, dr_t in [(wkv_sb, wkv_d), (wq_sb, wq_d),
                           (wo_sb, wo_d), (bkv_sb, bkv_d),
                           (bq_sb, bq_d), (bo_sb, bo_d), (ci_sb, ci_d),
                           (qi_sb, qi_d), (rl_sb, rl_d), (io_sb, io_d)]:
            nc.scalar.dma_start(sb_t[:], dr_t[:])
        nc.vector.memset(ones_sb[:], 1.0)
        make_identity(nc, ident[:])

        # ---- phase A: k/v for all nodes, q for own nodes ----
        with tc.tile_pool(name="xa", bufs=3) as xa, \
             tc.tile_pool(name="pa", bufs=2, space="PSUM") as pa:
            for i in range(XT):
                # SWDGE load with f32->f16 cast in flight; waits live on
                # the gpsimd engine, dodging the 1-wait HWDGE-load limit.
                xh = xa.tile([P, D], f16, name="xh")
                nc.gpsimd.dma_start(xh[:], x_d[i * P:(i + 1) * P, :])
                xT_ps = pa.tile([P, D], f16, name="xT_ps")
                nc.tensor.transpose(xT_ps[:], xh[:], ident[:])
                xT = xa.tile([P, D], f16, name="xT")
                nc.scalar.copy(xT[:], xT_ps[:])
                kv_ps = pa.tile([P, 2 * D], f32, name="kv_ps")
                nc.tensor.matmul(kv_ps[:], lhsT=ones_sb[:], rhs=bkv_sb[:],
                                 start=True, stop=False)
                nc.tensor.matmul(kv_ps[:], lhsT=xT[:], rhs=wkv_sb[:],
                                 start=False, stop=True)
                kv_sb = xa.tile([P, 2 * D], f16, name="kv_sb")
                nc.scalar.copy(kv_sb[:], kv_ps[:])
                nc.scalar.dma_start(kv_d[i * P:(i + 1) * P, :], kv_sb[:])

            for j in range(QT):
                xh2 = xa.tile([P, D], f16, name="xh2")
                nc.gpsimd.dma_start(xh2[:], xo_d[j * P:(j + 1) * P, :])
                xT2_ps = pa.tile([P, D], f16, name="xT2_ps")
                nc.tensor.transpose(xT2_ps[:], xh2[:], ident[:])
                xT2 = xa.tile([P, D], f16, name="xT2")
                nc.scalar.copy(xT2[:], xT2_ps[:])
                q_ps = pa.tile([P, D], f32, name="q_ps")
                nc.tensor.matmul(q_ps[:], lhsT=ones_sb[:], rhs=bq_sb[:],
                                 start=True, stop=False)
                nc.tensor.matmul(q_ps[:], lhsT=xT2[:], rhs=wq_sb[:],
                                 start=False, stop=True)
                q_sb = xa.tile([P, D], f16, name="q_sb")
                nc.scalar.copy(q_sb[:], q_ps[:])
                nc.scalar.dma_start(q_d[j * P:(j + 1) * P, :], q_sb[:])

        # ---- phase B: per-edge gather, scores, softmax, aggregation ----
        with tc.tile_pool(name="eg", bufs=3) as eg, \
             tc.tile_pool(name="ep", bufs=2, space="PSUM") as ep, \
             tc.tile_pool(name="yp", bufs=2, space="PSUM") as yp:
            for b in range(NBLK):
                ypre = yp.tile([P, D + H], f32, name="ypre")
                for t in range(TBLK):
                    T = b * TBLK + t
                    kv_g = eg.tile([P, 2 * D], f16, name="kv_g")
                    nc.gpsimd.indirect_dma_start(
                        out=kv_g[:], out_offset=None, in_=kv_d[:],
                        in_offset=bass.IndirectOffsetOnAxis(
                            ap=ci_sb[:, T:T + 1], axis=0))
                    q_g = eg.tile([P, D], f16, name="q_g")
                    nc.gpsimd.indirect_dma_start(
                        out=q_g[:], out_offset=None, in_=q_d[:],
                        in_offset=bass.IndirectOffsetOnAxis(
                            ap=qi_sb[:, T:T + 1], axis=0))
                    sel = eg.tile([P, P], bf16, name="sel")
                    nc.vector.tensor_tensor(
                        out=sel[:],
                        in0=rl_sb[:, T:T + 1].to_broadcast((P, P)),
                        in1=io_sb[:],
                        op=OP.is_equal)
                    prod = eg.tile([P, D], f32, name="prod")
                    nc.vector.tensor_tensor(out=prod[:], in0=q_g[:],
                                            in1=kv_g[:, 0:D], op=OP.mult)
                    s_t = eg.tile([P, H], f32, name="s_t")
                    nc.vector.tensor_reduce(
                        out=s_t[:],
                        in_=prod[:].rearrange("p (h d) -> p h d", h=H),
                        axis=mybir.AxisListType.X, op=OP.add)
                    wext = eg.tile([P, D + H], bf16, name="wext")
                    nc.scalar.activation(wext[:, D:D + H], s_t[:], AF.Exp)
                    for h in range(H):
                        nc.vector.tensor_tensor(
                            out=wext[:, h * DH:(h + 1) * DH],
                            in0=kv_g[:, D + h * DH:D + (h + 1) * DH],
                            in1=wext[:, D + h:D + h + 1].to_broadcast((P, DH)),
                            op=OP.mult)
                    nc.tensor.matmul(ypre[:], lhsT=sel[:], rhs=wext[:],
                                     start=(t == 0), stop=(t == TBLK - 1))

                zr = eg.tile([P, H], f32, name="zr")
                nc.vector.tensor_scalar_add(zr[:], ypre[:, D:D + H], 1e-30)
                rz = eg.tile([P, H], f32, name="rz")
                nc.vector.reciprocal(rz[:], zr[:])
                yb = eg.tile([P, D], f16, name="yb")
                for h in range(H):
                    nc.vector.tensor_tensor(
                        out=yb[:, h * DH:(h + 1) * DH],
                        in0=ypre[:, h * DH:(h + 1) * DH],
                        in1=rz[:, h:h + 1].to_broadcast((P, DH)),
                        op=OP.mult)
                yT_ps = ep.tile([P, D], f16, name="yT_ps")
                nc.tensor.transpose(yT_ps[:], yb[:], ident[:])
                yT = eg.tile([P, D], f16, name="yT")
                nc.scalar.copy(yT[:], yT_ps[:])
                o_ps = ep.tile([P, D], f32, name="o_ps")
                nc.tensor.matmul(o_ps[:], lhsT=ones_sb[:], rhs=bo_sb[:],
                                 start=True, stop=False)
                nc.tensor.matmul(o_ps[:], lhsT=yT[:], rhs=wo_sb[:],
                                 start=False, stop=True)
                o_sb = eg.tile([P, D], f32, name="o_sb")
                nc.scalar.copy(o_sb[:], o_ps[:])
                nc.scalar.dma_start(out_d[b * P:(b + 1) * P, :], o_sb[:])

    nc.compile()
    return nc


def _prepare_inputs(x, row, col, Wq, bq, Wk, bk, Wv, bv, Wo, bo,
                    NPAD, NOWN, NBLK, TBLK):
    """Host-side sharding: per-core padded edge lists + permuted weights."""
    N = x.shape[0]
    perm = _channel_perm()
    s = np.sqrt(float(H))
    wkv_in = np.ascontiguousarray(
        np.concatenate([Wk[perm, :].T, Wv[perm, :].T], axis=1)
    ).astype(np.float16)
    wq_in = np.ascontiguousarray((Wq[perm, :] / s).T).astype(np.float16)
    wo_in = np.ascontiguousarray(Wo[:, perm].T).astype(np.float16)
    bkv_in = np.concatenate([bk[perm], bv[perm]]).reshape(1, 2 * D).astype(np.float16)
    bq_in = (bq[perm] / s).reshape(1, D).astype(np.float16)
    bo_in = bo.reshape(1, D).astype(np.float16)
    io_in = np.tile(np.arange(P, dtype=np.float16), (P, 1))

    x_pad = np.zeros((NPAD, D), np.float32)
    x_pad[:N] = x

    NT = NBLK * TBLK
    EPC = NT * P  # padded edges per core
    in_maps = []
    for c in range(NCORES):
        lo, hi = c * NOWN, (c + 1) * NOWN
        e0 = np.searchsorted(row, lo, "left")
        e1 = np.searchsorted(row, hi, "left")
        rows_c = (row[e0:e1] - lo).astype(np.int64)
        cols_c = col[e0:e1].astype(np.int64)
        blk = rows_c // P
        # rank of each edge within its block (row-sorted ⇒ block-sorted)
        blk_starts = np.searchsorted(blk, np.arange(NBLK), "left")
        rank = np.arange(rows_c.shape[0]) - blk_starts[blk]
        cnts = np.bincount(blk, minlength=NBLK)
        if cnts.max() > TBLK * P:
            raise ValueError(f"TBLK={TBLK} too small: need "
                             f"{int(np.ceil(cnts.max() / P))}")
        pos = blk * (TBLK * P) + rank
        ci = np.zeros(EPC, np.int32)
        qi = np.zeros(EPC, np.int32)
        rl = np.full(EPC, -1.0, np.float16)
        ci[pos] = cols_c.astype(np.int32)
        qi[pos] = rows_c.astype(np.int32)
        rl[pos] = (rows_c % P).astype(np.float16)
        in_maps.append({
            "x": x_pad, "xo": np.ascontiguousarray(x_pad[lo:hi]),
            "wkv": wkv_in, "wq": wq_in, "wo": wo_in,
            "bkv": bkv_in, "bq": bq_in, "bo": bo_in,
            "ci": np.ascontiguousarray(ci.reshape(NT, P).T),
            "qi": np.ascontiguousarray(qi.reshape(NT, P).T),
            "rl": np.ascontiguousarray(rl.reshape(NT, P).T),
            "io": io_in,
        })
    return in_maps


def _required_tblk(row, NOWN, NBLK):
    row = np.asarray(row, np.int64)
    need = 1
    for c in range(NCORES):
        lo, hi = c * NOWN, (c + 1) * NOWN
        e0 = np.searchsorted(row, lo, "left")
        e1 = np.searchsorted(row, hi, "left")
        blk = (row[e0:e1] - lo) // P
        cnts = np.bincount(blk, minlength=NBLK)
        need = max(need, int(np.ceil(cnts.max() / P)))
    return need


def _install_ntff_hook():
    """The agent image's antenv lacks axon_hooks; inject it so trace=True
    can drive NTFF profiling through libaxon_pjrt.so."""
    import importlib
    try:
        importlib.import_module("antenv.axon_hooks")
        return
    except ImportError:
        pass
    import types
    if "/root/.axon_site" not in sys.path:
        sys.path.insert(0, "/root/.axon_site")
    from trn_agent_boot.trn_boot import _ntff_profile_via_ctypes
    hook = _ntff_profile_via_ctypes("/opt/axon/libaxon_pjrt.so")
    mod = types.ModuleType("antenv.axon_hooks")
    state = {"hook": hook}
    mod.get_axon_ntff_profile_hook = lambda: state["hook"]
    mod.set_axon_ntff_profile_hook = lambda h: state.update(hook=h)
    import antenv
    antenv.axon_hooks = mod
    sys.modules["antenv.axon_hooks"] = mod


def run(x, row, col, Wq, bq, Wk, bk, Wv, bv, Wo, bo, NBLK=NBLK_FULL,
        trace=False, tmpdir=None):
    from concourse import bass_utils
    from concourse.bass_utils import run_bass_kernel_spmd
    if trace:
        _install_ntff_hook()
        bass_utils.upload_artifacts = lambda d: "local://" + d

    x = np.asarray(x, np.float32)
    row = np.asarray(row, np.int64)
    col = np.asarray(col, np.int64)
    N = x.shape[0]
    NOWN = NBLK * P
    NPAD = NCORES * NOWN
    assert NPAD >= N
    TBLK = _required_tblk(row, NOWN, NBLK)
    nc = _build_program(NPAD, NOWN, NBLK, TBLK)
    in_maps = _prepare_inputs(
        x, row, col,
        np.asarray(Wq, np.float32), np.asarray(bq, np.float32),
        np.asarray(Wk, np.float32), np.asarray(bk, np.float32),
        np.asarray(Wv, np.float32), np.asarray(bv, np.float32),
        np.asarray(Wo, np.float32), np.asarray(bo, np.float32),
        NPAD, NOWN, NBLK, TBLK)
    res = run_bass_kernel_spmd(nc, in_maps, list(range(NCORES)), trace=trace,
                           tmpdir=tmpdir)
    out = np.concatenate([res.results[c]["out"] for c in range(NCORES)], 0)
    return out[:N].astype(np.float32), res


def kernel(**inputs):
    out, _ = run(**inputs)
    return out
